# revision 79
# baseline (speedup 1.0000x reference)
"""Causal self-attention (B=4, T=2048, C=1024, NH=16) on 8 TRN2 NeuronCores.

Sharding (tensor-parallel heads x data-parallel batch):
  - 4 core-pairs: pair p = cores (2p, 2p+1) handles batch b = p.
  - Within a pair, rank 0 computes heads 0-7, rank 1 heads 8-15
    (w_qkv output columns split by head group).
  - Each core computes a FULL-width partial projection over its own heads
    (w_proj row-split per the tensor-parallel scheme); the host sums the
    two partials per pair during unshard (the all-reduce of the hint),
    so the device program needs no collective at all.

Device algorithm (per core):
  Phase 1  QKV GEMMs in fp16 (fp32 PSUM): q/k produced d-major
           [c_out 128, T], v t-major with a fused ones-column that makes
           a@V also accumulate the softmax denominator.
  Phase 2  Attention per (head-pair j, 512-wide q block): causal-chunked
           scores sT[kv,q] on the PE (two heads packed in partition
           ranges 0-63/64-127), exp on ScalarE (scores ~N(0,1): no max
           subtraction needed), static-triangle mask multiply on the
           diagonal chunk, then q-major a@V: out[q 128, d 65] accumulated
           per 128-q subblock (M=128 instead of the d-major M=65 — half
           the PE rows). start=True zeroes a whole 2KB PSUM bank, so only
           the first matmul into each fresh aug bank carries it.
           Normalization: one small DVE copy stages the finished subblock
           out of PSUM (keeps the aug-tile WAR window tiny), GpSimd
           normalize_recip divides by the denominator column, and an
           SBUF->SBUF XBAR DMA transpose flips the tile back to d-major
           attnT layout in ci_sb.
  Phase 3  Projection y_part[t, 1024] = sum_j attnT_j.T @ wp_j straight
           from the SBUF-resident ci tiles, drained via DVE to fp16 and
           DMA'd out.

Scheduling: the Tile scheduler is an out-of-order per-engine list
scheduler, so all GEMM work (v, q/k of later pairs, projection blocks)
is emitted as "filler units" through a deque with just-in-time forcing
markers; attention instructions run at priority 0 so exp never starves,
fillers are pinned to attention progress with artificial deps, and the
projection is gated per completed q-block of the last pair. Everything
off-PSUM is fp16 (halves DMA bytes and lifts the fp32r free-dim>=256
restriction); PSUM stays fp32 and uses exactly 8 banks.
"""

import numpy as np
from collections import deque

import concourse.bass as bass
import concourse.mybir as mybir
import concourse.tile as tile
from concourse.tile import add_dep_helper
from concourse import bacc
from concourse.bass_utils import run_bass_kernel_spmd

B, T, C = 4, 2048, 1024
NH, HD = 16, 64
N_CORES = 8
HPC = NH // 2          # heads per core
NPAIR = HPC // 2       # head-pairs per core
TB = T // 128          # 128-row t blocks
QBS = T // 512         # 512-wide q blocks
KC = C // 128          # 128-deep contraction chunks for qkv/proj
SCALE = float(1.0 / np.sqrt(HD))

F32 = mybir.dt.float32
F16 = mybir.dt.float16
AF = mybir.ActivationFunctionType
REPLICA_GROUPS = [[0, 1], [2, 3], [4, 5], [6, 7]]

# cost-model constants used only to pace filler emission (ns)
PE_ROW = 0.4167
ACT_ROW = 0.8333
ACT_FIX = 400.0


def build_nc(reps=1, single_core=False):
    nc = bacc.Bacc(
        "TRN2", target_bir_lowering=False, debug=False,
        num_devices=(1 if single_core else N_CORES),
    )

    xt = nc.dram_tensor("xt", [C, T], F16, kind="ExternalInput")
    wq = nc.dram_tensor("wq", [C, 512], F16, kind="ExternalInput")
    wk = nc.dram_tensor("wk", [C, 512], F16, kind="ExternalInput")
    wv = nc.dram_tensor("wv", [C, 512], F16, kind="ExternalInput")
    wp = nc.dram_tensor("wp", [512, C], F16, kind="ExternalInput")
    mask = nc.dram_tensor("mask", [128, 2, 128], F16, kind="ExternalInput")
    y = nc.dram_tensor("y", [T, C], F16, kind="ExternalOutput")

    with tile.TileContext(nc) as tc:
        for _rep in range(reps):
            _emit_one(nc, tc, xt, wq, wk, wv, wp, mask, y, single_core)

    nc.compile()
    return nc


def _emit_one(nc, tc, xt, wq, wk, wv, wp, mask, y, single_core):
    with tc.tile_pool(name="persist", bufs=1) as persist, \
         tc.tile_pool(name="xtp", bufs=1) as xtp, \
         tc.tile_pool(name="wvp", bufs=1) as wvp, \
         tc.tile_pool(name="wqk", bufs=4) as wqkp, \
         tc.tile_pool(name="qp", bufs=2) as qp, \
         tc.tile_pool(name="atp", bufs=8) as atp, \
         tc.tile_pool(name="rcpp", bufs=6) as rcpp, \
         tc.tile_pool(name="nbp", bufs=6) as nbp, \
         tc.tile_pool(name="cisp", bufs=NPAIR) as cisp, \
         tc.tile_pool(name="partp", bufs=8) as partp, \
         tc.tile_pool(name="ysbp", bufs=6) as ysbp, \
         tc.tile_pool(name="psq", bufs=2, space="PSUM") as psq, \
         tc.tile_pool(name="ps2", bufs=2, space="PSUM") as ps2, \
         tc.tile_pool(name="paug", bufs=1, space="PSUM") as paug:

        kT_sb = persist.tile([128, NPAIR, T], F16)
        v_sb = persist.tile([128, TB, HPC, HD + 1], F16)
        wp_sb = persist.tile([128, NPAIR, C], F16)
        mask_sb = persist.tile([128, 2, 128], F16)
        xt_sb = xtp.tile([128, KC, T], F16)
        wv_sb = wvp.tile([128, KC, 512], F16)

        xt_r = xt[:].rearrange("(a p) t -> p a t", p=128)
        wv_r = wv[:].rearrange("(a p) n -> p a n", p=128)
        wq_r = wq[:].rearrange("(a p) n -> p a n", p=128)
        wk_r = wk[:].rearrange("(a p) n -> p a n", p=128)
        wpr = wp[:].rearrange("(j p) n -> p j n", p=128)

        # upfront DMAs: phase-1 needs full-depth xt, but the first t-block
        # work (v tb0-3, q/k i0) reads only cols 0:512 of each chunk — load
        # those first so the PE starts ~3.5us in, then stream the rest
        nc.sync.dma_start(out=wv_sb[:], in_=wv_r[:])
        nc.sync.dma_start(out=xt_sb[:, 0, 0:512], in_=xt_r[:, 0, 0:512])
        for a in range(1, KC):
            nc.sync.dma_start(out=xt_sb[:, a, 0:512], in_=xt_r[:, a, 0:512])
        wq_tiles, wk_tiles = {}, {}

        def fetch_wqk(j):
            if j in wq_tiles or j >= NPAIR:
                return
            wq_tiles[j] = wqkp.tile([128, KC, 128], F16, tag="wq", name=f"wq{j}")
            wk_tiles[j] = wqkp.tile([128, KC, 128], F16, tag="wk", name=f"wk{j}")
            nc.sync.dma_start(
                out=wq_tiles[j][:], in_=wq_r[:, :, 128 * j:128 * (j + 1)])
            nc.sync.dma_start(
                out=wk_tiles[j][:], in_=wk_r[:, :, 128 * j:128 * (j + 1)])

        fetch_wqk(0)
        nc.sync.dma_start(out=mask_sb[:], in_=mask[:])
        for a in range(KC):
            nc.sync.dma_start(out=xt_sb[:, a, 512:T], in_=xt_r[:, a, 512:T])
        nc.sync.dma_start(out=wp_sb[:], in_=wpr[:])
        nc.vector.memset(v_sb[:, :, :, HD], 1.0)

        q_tiles = {}
        ci_tiles = {}
        anchor = [None]  # most recent attention score mm (filler pacing)

        def pace(inst):
            if anchor[0] is not None:
                add_dep_helper(inst.ins, anchor[0], sync=True,
                               reason="filler paced to attention progress")
        # co-simulated engine clocks (ns) used to place filler work so the
        # PE never out-runs ScalarE's exp stream
        clk = {"pe": 0.0, "act": 0.0, "att3_cols": 0, "chunk": 0}

        # ---------------- filler units (PE-feeding work) ----------------
        def v_unit(tb):
            def emit():
                ps = psq.tile([128, 512], F32, tag="acc", name="psv")
                for a in range(KC):
                    m = nc.tensor.matmul(
                        ps[:], xt_sb[:, a, 128 * tb:128 * (tb + 1)],
                        wv_sb[:, a, :], start=(a == 0), stop=(a == KC - 1))
                    if a == 0:
                        pace(m)
                nc.vector.tensor_copy(
                    v_sb[:, tb, :, 0:HD],
                    ps[:].rearrange("p (h d) -> p h d", h=HPC))
            return ("v", tb, emit, 8 * 512 * PE_ROW)

        # scheduler time-fences (ms of sim time): hold each pair's q/k
        # GEMMs in reserve so the out-of-order tile scheduler doesn't burn
        # them early — they are the PE filler for the ACT-paced attention
        # window of the preceding pair
        QK_FENCE = {0: 0.0, 1: 0.0, 2: 0.0, 3: 0.0}

        def qk_unit(j, which, i):
            def emit():
                if which == "q" and j not in q_tiles:
                    q_tiles[j] = qp.tile([128, T], F16, tag="q",
                                         name=f"q{j}")
                wsb = wq_tiles[j] if which == "q" else wk_tiles[j]
                with tc.tile_wait_until(QK_FENCE[j], enable=QK_FENCE[j] > 0):
                    ps = psq.tile([128, 512], F32, tag="acc", name="psqk")
                    for a in range(KC):
                        m = nc.tensor.matmul(
                            ps[:], wsb[:, a, :],
                            xt_sb[:, a, 512 * i:512 * (i + 1)],
                            start=(a == 0), stop=(a == KC - 1))
                        if a == 0:
                            pace(m)
                    if which == "q":
                        nc.vector.tensor_scalar_mul(
                            q_tiles[j][:, 512 * i:512 * (i + 1)], ps[:], SCALE)
                    else:
                        nc.vector.tensor_copy(
                            kT_sb[:, j, 512 * i:512 * (i + 1)], ps[:])
            return ("qk", j, emit, 8 * 512 * PE_ROW)

        y_r = y[:].rearrange("(t p) n -> p t n", p=128)
        part_tiles = {}

        def partial_unit(tb, half):
            # pairs 0-2 of the K accumulation: ready as soon as att(2) has
            # produced these columns, so these are clean att(2)/att(3) fillers
            def emit():
                ps = psq.tile([128, 512], F32, tag="acc", name="psp")
                for j in range(3):
                    m = nc.tensor.matmul(
                        ps[:], ci_tiles[j][:, 128 * tb:128 * (tb + 1)],
                        wp_sb[:, j, 512 * half:512 * (half + 1)],
                        start=(j == 0), stop=(j == 2))
                    if j == 0:
                        pace(m)
                part_tiles[(tb, half)] = partp.tile(
                    [128, 512], F16, tag="part", name=f"part{tb}_{half}")
                nc.vector.tensor_copy(part_tiles[(tb, half)][:], ps[:])
            return ("part", tb, emit, 3 * 512 * PE_ROW)

        def proj_unit(tb, half):
            def emit():
                # late units borrow a (by then idle) score-pool bank so four
                # accumulation groups can be in flight instead of two
                if tb >= 12 and (tb + half) % 2 == 1:
                    ps2t = ps2.tile([128, 2, 512], F32, tag="s2", name="psp2")
                    ps = ps2t[:, 0, :]
                else:
                    ps = psq.tile([128, 512], F32, tag="acc", name="psj")[:]
                for j in range(NPAIR):
                    m = nc.tensor.matmul(
                        ps, ci_tiles[j][:, 128 * tb:128 * (tb + 1)],
                        wp_sb[:, j, 512 * half:512 * (half + 1)],
                        start=(j == 0), stop=(j == NPAIR - 1))
                    if j == 0:
                        pace(m)
                ysb = ysbp.tile([128, 512], F16, tag="ysb", name="ysb")
                nc.vector.tensor_copy(ysb[:], ps)
                yeng = nc.scalar if tb >= 12 else nc.sync
                yeng.dma_start(
                    out=y_r[:, tb, 512 * half:512 * (half + 1)], in_=ysb[:])
            return ("proj", tb, emit, NPAIR * 512 * PE_ROW)

        def finish_unit(tb, half):
            def emit():
                ps = psq.tile([128, 512], F32, tag="acc", name="psf")
                m = nc.tensor.matmul(
                    ps[:], ci_tiles[3][:, 128 * tb:128 * (tb + 1)],
                    wp_sb[:, 3, 512 * half:512 * (half + 1)],
                    start=True, stop=True)
                pace(m)
                ysb = ysbp.tile([128, 512], F16, tag="ysb", name="ysb")
                nc.vector.tensor_add(
                    ysb[:], part_tiles[(tb, half)][:], ps[:])
                nc.sync.dma_start(
                    out=y_r[:, tb, 512 * half:512 * (half + 1)], in_=ysb[:])
            return ("fin", tb, emit, 512 * PE_ROW)

        fillers = deque()
        markers = {}
        for i in range(QBS):
            for tb in range(4 * i, 4 * (i + 1)):
                fillers.append(v_unit(tb))
            fillers.append(qk_unit(0, "q", i))
            fillers.append(qk_unit(0, "k", i))
            markers[(0, i)] = len(fillers)
        for j in range(1, NPAIR):
            for i in range(QBS):
                fillers.append(qk_unit(j, "q", i))
                fillers.append(qk_unit(j, "k", i))
                markers[(j, i)] = len(fillers)
        for tb in range(TB):
            for half in range(2):
                fillers.append(proj_unit(tb, half))
        n_popped = [0]

        def head_legal():
            kind, tb, _, _ = fillers[0]
            if kind == "qk" and tb not in wq_tiles:
                return False  # pair weights not prefetched yet
            if kind == "proj":
                # proj tb needs every pair's attnT columns for its q-block
                return (tb // 4) in clk["att3_done"]
            return True

        def pop_one():
            _, _, emit, pe_ns = fillers.popleft()
            emit()
            n_popped[0] += 1
            clk["pe"] += pe_ns

        def pop_until(target, cap=12):
            n = 0
            while fillers and n < cap and clk["pe"] < target and head_legal():
                pop_one()
                n += 1

        def force_through(marker):
            while n_popped[0] < marker:
                pop_one()

        # ---------------- attention ----------------
        EXP_LAT = 300.0

        def att_qb(j, qb):
            aug = paug.tile([128, 2, 4, 128], F32, tag="aug", name="aug")
            nchunks = 4 * (qb + 1)
            pending = None
            pending_ready = 0.0
            for c in range(nchunks):
                diag = c >= 4 * qb
                o = 128 * (c - 4 * qb) if diag else 0
                s0 = c - 4 * qb
                s2 = ps2.tile([128, 2, 512], F32, tag="s2", name="s2")
                with tc.high_priority():
                    for hh in range(2):
                        m = nc.tensor.matmul(
                            s2[:, hh, o:512],
                            kT_sb[64 * hh:64 * hh + 64, j,
                                  128 * c:128 * (c + 1)],
                            q_tiles[j][64 * hh:64 * hh + 64,
                                       512 * qb + o:512 * (qb + 1)],
                            start=True, stop=True)
                        if hh == 0:
                            anchor[0] = m.ins
                    rows = 2 * (512 - o)
                    clk["pe"] += rows * PE_ROW
                    aT = atp.tile([128, 2, 512], F16, tag="aT", name="aT")
                    nc.scalar.activation(aT[:, :, o:512], s2[:, :, o:512],
                                         AF.Exp)
                    clk["act"] = max(clk["act"], clk["pe"] + EXP_LAT) \
                        + rows * ACT_ROW + ACT_FIX
                    if diag:
                        nc.vector.tensor_mul(
                            aT[:, :, o:o + 128], aT[:, :, o:o + 128],
                            mask_sb[:])
                # flush the previous chunk's a@V now that this chunk's scores
                # are in flight; insert fillers if the PE would beat the exp
                if pending is not None:
                    clk["chunk"] += 1
                    cad = 1 if j == 3 else 5
                    if clk["chunk"] % cad == 0:
                        pop_until(clk["pe"] + 1.0, cap=2)
                    pending()
                pending = _make_av(j, qb, c, diag, s0, aug, aT)
                pending_ready = clk["act"]
            pending()

        def _make_av(j, qb, c, diag, s0, aug, aT):
            def emit_av():
                # masked diagonal subblock s0 goes last so the other a@V
                # matmuls don't sit behind the DVE mask in PE order.
                # start=True zeroes the whole 2KB PSUM bank (zero region), so
                # only the FIRST matmul touching each hh-bank of a fresh aug
                # tile carries it; siblings are ordered after the zeroing and
                # accumulate onto zeros.
                subs = list(range(max(0, s0) + 1, 4)) + [max(0, s0)]
                stk3 = tc.high_priority()
                stk3.__enter__()
                for hh in range(2):
                    clear_mm = None
                    for s in subs:
                        mm = nc.tensor.matmul(
                            aug[:, hh, s, 0:HD + 1],
                            aT[:, hh, 128 * s:128 * (s + 1)],
                            v_sb[:, c, 2 * j + hh, :],
                            start=(c == 0 and clear_mm is None),
                            stop=(c == 4 * qb + s),
                            skip_group_check=True)
                        if c == 0:
                            if clear_mm is None:
                                clear_mm = mm
                            else:
                                add_dep_helper(
                                    mm.ins, clear_mm.ins, sync=True,
                                    reason="PSUM zero-region ordering")
                clk["pe"] += len(subs) * 2 * 65 * PE_ROW
                stk3.__exit__(None, None, None)
                if diag:
                    # stage the finished subblock out of PSUM with one small
                    # DVE copy (shrinks the aug-tile WAR window to ~260ns),
                    # then normalize on the otherwise-idle GpSimd engine
                    with tc.high_priority():
                        stg = rcpp.tile([128, 2, HD + 1], F32, tag="stg",
                                        name="stg")
                        nc.vector.tensor_copy(stg[:], aug[:, :, s0, 0:HD + 1])
                    nb = nbp.tile([128, 2, HD], F16, tag="nb", name="nb")
                    for hh in range(2):
                        nc.gpsimd.normalize_recip(
                            nb[:, hh, :], stg[:, hh, 0:HD],
                            stg[:, hh, HD:HD + 1])
                    # the last pair's late transposes go out on the
                    # Activation engine's DMA queue: its exp stream is done
                    # by then, and this keeps their semaphore waits from
                    # head-of-line blocking the projection drains' y-DMAs
                    # behind them in SP's in-order issue queue
                    eng = nc.scalar if (j == 3 and qb >= 2) else nc.sync
                    eng.dma_start_transpose(
                        ci_sb[:, 512 * qb + 128 * s0:
                              512 * qb + 128 * (s0 + 1)],
                        nb[:])
            return emit_av

        for j in range(NPAIR):
            ci_sb = cisp.tile([128, T], F16, tag="ci", name=f"ci{j}")
            ci_tiles[j] = ci_sb
            fetch_wqk(j + 1)
            fetch_wqk(j + 2)
            fetch_wqk(j + 3)
            clk["pe"] = clk["act"] = 0.0
            for qb in range(QBS):
                force_through(markers[(j, qb)])
                att_qb(j, qb)
                if j == 1:
                    clk["att1_cols"] = 512 * (qb + 1)
                if j == 3:
                    clk["att3_done"].add(qb)

        while fillers:
            pop_one()


_NC_CACHE = None


def _get_nc():
    global _NC_CACHE
    if _NC_CACHE is None:
        _NC_CACHE = build_nc()
    return _NC_CACHE


def _mask_np():
    # mask[kv', hh, q'] = 1 where q' >= kv' (within-chunk causal triangle),
    # duplicated over the two heads packed per score tile
    kv = np.arange(128)[:, None]
    q = np.arange(128)[None, :]
    tri = (q >= kv).astype(np.float16)
    return np.ascontiguousarray(
        np.broadcast_to(tri[:, None, :], (128, 2, 128)))


def shard_inputs(x, w_qkv, w_proj):
    x = np.asarray(x, dtype=np.float16)
    w_qkv = np.asarray(w_qkv, dtype=np.float16)
    w_proj = np.asarray(w_proj, dtype=np.float16)
    mask = _mask_np()
    in_maps = []
    for core in range(N_CORES):
        pair, rank = divmod(core, 2)
        c0 = HD * HPC * rank  # 0 or 512: this core's head-column offset
        in_maps.append({
            "xt": np.ascontiguousarray(x[pair].T),
            "wq": np.ascontiguousarray(w_qkv[:, c0:c0 + 512]),
            "wk": np.ascontiguousarray(w_qkv[:, C + c0:C + c0 + 512]),
            "wv": np.ascontiguousarray(w_qkv[:, 2 * C + c0:2 * C + c0 + 512]),
            "wp": np.ascontiguousarray(w_proj[:, 512 * rank:512 * rank + 512]),
            "mask": mask,
        })
    return in_maps


def assemble_output(results):
    out = np.empty((B, T, C), dtype=np.float32)
    for core in range(N_CORES):
        pair, rank = divmod(core, 2)
        out[pair][:, 512 * rank:512 * rank + 512] = \
            results[core]["y"].astype(np.float32)
    return out


# --- cached PJRT runner (same path run_bass_kernel_spmd takes under axon,
# but keeps the jitted executable so repeat calls skip re-tracing) ---
_RUNNER_CACHE = None


def _make_runner(nc):
    import jax
    import numpy as _np
    from jax.sharding import Mesh, PartitionSpec
    from jax.experimental.shard_map import shard_map
    from concourse import bass2jax
    from concourse.bass2jax import _bass_exec_p, install_neuronx_cc_hook

    install_neuronx_cc_hook()
    part_name = (nc.partition_id_tensor.name
                 if nc.partition_id_tensor else None)
    in_names, out_names, out_avals, zero_shapes = [], [], [], []
    for alloc in nc.m.functions[0].allocations:
        if not isinstance(alloc, mybir.MemoryLocationSet):
            continue
        name = alloc.memorylocations[0].name
        if alloc.kind == "ExternalInput":
            if name != part_name:
                in_names.append(name)
        elif alloc.kind == "ExternalOutput":
            out_names.append(name)
            shape = tuple(alloc.tensor_shape)
            dtype = mybir.dt.np(alloc.dtype)
            out_avals.append(jax.core.ShapedArray(shape, dtype))
            zero_shapes.append((shape, dtype))
    n_params = len(in_names)
    n_outs = len(out_names)
    all_in_names = in_names + out_names
    if part_name is not None:
        all_in_names = all_in_names + [part_name]

    def _body(*args):
        operands = list(args)
        if part_name is not None:
            operands.append(bass2jax.partition_id_tensor())
        outs = _bass_exec_p.bind(
            *operands,
            out_avals=tuple(out_avals),
            in_names=tuple(all_in_names),
            out_names=tuple(out_names),
            lowering_input_output_aliases=(),
            sim_require_finite=True,
            sim_require_nnan=True,
            nc=nc,
        )
        return tuple(outs)

    devices = jax.devices()[:N_CORES]
    mesh = Mesh(_np.asarray(devices), ("core",))
    in_specs = (PartitionSpec("core"),) * (n_params + n_outs)
    out_specs = (PartitionSpec("core"),) * n_outs
    donate = tuple(range(n_params, n_params + n_outs))
    sharded = jax.jit(
        shard_map(_body, mesh=mesh, in_specs=in_specs, out_specs=out_specs,
                  check_rep=False),
        donate_argnums=donate, keep_unused=True,
    )

    def run(in_maps):
        concat_in = [
            _np.concatenate([_np.asarray(in_maps[c][nm]) for c in
                             range(N_CORES)], axis=0)
            for nm in in_names
        ]
        concat_zeros = [
            _np.zeros((N_CORES * s[0], *s[1:]), d) for s, d in zero_shapes
        ]
        out_arrs = sharded(*concat_in, *concat_zeros)
        return [
            {nm: _np.asarray(out_arrs[i]).reshape(
                N_CORES, *out_avals[i].shape)[c]
             for i, nm in enumerate(out_names)}
            for c in range(N_CORES)
        ]

    run.sharded = sharded
    run.in_names = in_names
    run.zero_shapes = zero_shapes
    run.mesh = mesh
    return run


def _get_runner():
    global _RUNNER_CACHE
    if _RUNNER_CACHE is None:
        _RUNNER_CACHE = _make_runner(_get_nc())
    return _RUNNER_CACHE


def kernel(x, w_qkv, w_proj):
    in_maps = shard_inputs(x, w_qkv, w_proj)
    try:
        results = _get_runner()(in_maps)
    except Exception:
        res = run_bass_kernel_spmd(_get_nc(), in_maps, list(range(N_CORES)))
        results = res.results
    return assemble_output(results)


# revision 80
# speedup vs baseline: 1.0720x; 1.0720x over previous
"""Causal self-attention (B=4, T=2048, C=1024, NH=16) on 8 TRN2 NeuronCores.

Sharding (tensor-parallel heads x data-parallel batch):
  - 4 core-pairs: pair p = cores (2p, 2p+1) handles batch b = p.
  - Within a pair, rank 0 computes heads 0-7, rank 1 heads 8-15
    (w_qkv output columns split by head group).
  - Each core computes a FULL-width partial projection over its own heads
    (w_proj row-split per the tensor-parallel scheme); the host sums the
    two partials per pair during unshard (the all-reduce of the hint),
    so the device program needs no collective at all.

Device algorithm (per core):
  Phase 1  QKV GEMMs in fp16 (fp32 PSUM): q/k produced d-major
           [c_out 128, T], v t-major with a fused ones-column that makes
           a@V also accumulate the softmax denominator.
  Phase 2  Attention per (head-pair j, 512-wide q block): causal-chunked
           scores sT[kv,q] on the PE (two heads packed in partition
           ranges 0-63/64-127), exp on ScalarE (scores ~N(0,1): no max
           subtraction needed), static-triangle mask multiply on the
           diagonal chunk, then q-major a@V: out[q 128, d 65] accumulated
           per 128-q subblock (M=128 instead of the d-major M=65 — half
           the PE rows). start=True zeroes a whole 2KB PSUM bank, so only
           the first matmul into each fresh aug bank carries it.
           Normalization: one small DVE copy stages the finished subblock
           out of PSUM (keeps the aug-tile WAR window tiny), GpSimd
           normalize_recip divides by the denominator column, and an
           SBUF->SBUF XBAR DMA transpose flips the tile back to d-major
           attnT layout in ci_sb.
  Phase 3  Projection y_part[t, 1024] = sum_j attnT_j.T @ wp_j straight
           from the SBUF-resident ci tiles, drained via DVE to fp16 and
           DMA'd out.

Scheduling: the Tile scheduler is an out-of-order per-engine list
scheduler, so all GEMM work (v, q/k of later pairs, projection blocks)
is emitted as "filler units" through a deque with just-in-time forcing
markers; attention instructions run at priority 0 so exp never starves,
fillers are pinned to attention progress with artificial deps, and the
projection is gated per completed q-block of the last pair. Everything
off-PSUM is fp16 (halves DMA bytes and lifts the fp32r free-dim>=256
restriction); PSUM stays fp32 and uses exactly 8 banks.
"""

import numpy as np
from collections import deque

import concourse.bass as bass
import concourse.mybir as mybir
import concourse.tile as tile
from concourse.tile import add_dep_helper
from concourse import bacc
from concourse.bass_utils import run_bass_kernel_spmd

B, T, C = 4, 2048, 1024
NH, HD = 16, 64
N_CORES = 8
HPC = NH // 2          # heads per core
NPAIR = HPC // 2       # head-pairs per core
TB = T // 128          # 128-row t blocks
QBS = T // 512         # 512-wide q blocks
KC = C // 128          # 128-deep contraction chunks for qkv/proj
SCALE = float(1.0 / np.sqrt(HD))

F32 = mybir.dt.float32
F16 = mybir.dt.float16
AF = mybir.ActivationFunctionType
REPLICA_GROUPS = [[0, 1], [2, 3], [4, 5], [6, 7]]

# cost-model constants used only to pace filler emission (ns)
PE_ROW = 0.4167
ACT_ROW = 0.8333
ACT_FIX = 240.0


def build_nc(reps=1, single_core=False):
    nc = bacc.Bacc(
        "TRN2", target_bir_lowering=False, debug=False,
        num_devices=(1 if single_core else N_CORES),
    )

    xt = nc.dram_tensor("xt", [C, T], F16, kind="ExternalInput")
    wq = nc.dram_tensor("wq", [C, 512], F16, kind="ExternalInput")
    wk = nc.dram_tensor("wk", [C, 512], F16, kind="ExternalInput")
    wv = nc.dram_tensor("wv", [C, 512], F16, kind="ExternalInput")
    wp = nc.dram_tensor("wp", [512, C], F16, kind="ExternalInput")
    mask = nc.dram_tensor("mask", [128, 2, 128], F16, kind="ExternalInput")
    y = nc.dram_tensor("y", [T, C], F16, kind="ExternalOutput")

    with tile.TileContext(nc) as tc:
        for _rep in range(reps):
            _emit_one(nc, tc, xt, wq, wk, wv, wp, mask, y, single_core)

    nc.compile()
    return nc


def _emit_one(nc, tc, xt, wq, wk, wv, wp, mask, y, single_core):
    with tc.tile_pool(name="persist", bufs=1) as persist, \
         tc.tile_pool(name="xtp", bufs=1) as xtp, \
         tc.tile_pool(name="wvp", bufs=1) as wvp, \
         tc.tile_pool(name="wqk", bufs=4) as wqkp, \
         tc.tile_pool(name="qp", bufs=2) as qp, \
         tc.tile_pool(name="atp", bufs=8) as atp, \
         tc.tile_pool(name="rcpp", bufs=6) as rcpp, \
         tc.tile_pool(name="nbp", bufs=6) as nbp, \
         tc.tile_pool(name="cisp", bufs=NPAIR) as cisp, \
         tc.tile_pool(name="partp", bufs=8) as partp, \
         tc.tile_pool(name="ysbp", bufs=6) as ysbp, \
         tc.tile_pool(name="psq", bufs=2, space="PSUM") as psq, \
         tc.tile_pool(name="ps2", bufs=2, space="PSUM") as ps2, \
         tc.tile_pool(name="paug", bufs=1, space="PSUM") as paug:

        kT_sb = persist.tile([128, NPAIR, T], F16)
        v_sb = persist.tile([128, TB, HPC, HD + 1], F16)
        wp_sb = persist.tile([128, NPAIR, C], F16)
        mask_sb = persist.tile([128, 2, 128], F16)
        xt_sb = xtp.tile([128, KC, T], F16)
        wv_sb = wvp.tile([128, KC, 512], F16)

        xt_r = xt[:].rearrange("(a p) t -> p a t", p=128)
        wv_r = wv[:].rearrange("(a p) n -> p a n", p=128)
        wq_r = wq[:].rearrange("(a p) n -> p a n", p=128)
        wk_r = wk[:].rearrange("(a p) n -> p a n", p=128)
        wpr = wp[:].rearrange("(j p) n -> p j n", p=128)

        # upfront DMAs: phase-1 needs full-depth xt, but the first t-block
        # work (v tb0-3, q/k i0) reads only cols 0:512 of each chunk — load
        # those first so the PE starts ~3.5us in, then stream the rest
        nc.sync.dma_start(out=wv_sb[:], in_=wv_r[:])
        nc.sync.dma_start(out=xt_sb[:, 0, 0:512], in_=xt_r[:, 0, 0:512])
        for a in range(1, KC):
            nc.sync.dma_start(out=xt_sb[:, a, 0:512], in_=xt_r[:, a, 0:512])
        wq_tiles, wk_tiles = {}, {}

        def fetch_wqk(j):
            if j in wq_tiles or j >= NPAIR:
                return
            wq_tiles[j] = wqkp.tile([128, KC, 128], F16, tag="wq", name=f"wq{j}")
            wk_tiles[j] = wqkp.tile([128, KC, 128], F16, tag="wk", name=f"wk{j}")
            nc.sync.dma_start(
                out=wq_tiles[j][:], in_=wq_r[:, :, 128 * j:128 * (j + 1)])
            nc.sync.dma_start(
                out=wk_tiles[j][:], in_=wk_r[:, :, 128 * j:128 * (j + 1)])

        fetch_wqk(0)
        nc.sync.dma_start(out=mask_sb[:], in_=mask[:])
        for a in range(KC):
            nc.sync.dma_start(out=xt_sb[:, a, 512:T], in_=xt_r[:, a, 512:T])
        nc.sync.dma_start(out=wp_sb[:], in_=wpr[:])
        nc.vector.memset(v_sb[:, :, :, HD], 1.0)

        q_tiles = {}
        ci_tiles = {}
        anchor = [None]  # most recent attention score mm (filler pacing)

        def pace(inst):
            if anchor[0] is not None:
                add_dep_helper(inst.ins, anchor[0], sync=True,
                               reason="filler paced to attention progress")
        # co-simulated engine clocks (ns) used to place filler work so the
        # PE never out-runs ScalarE's exp stream
        clk = {"pe": 0.0, "act": 0.0, "att3_cols": 0, "chunk": 0}

        # ---------------- filler units (PE-feeding work) ----------------
        def v_unit(tb):
            def emit():
                ps = psq.tile([128, 512], F32, tag="acc", name="psv")
                for a in range(KC):
                    m = nc.tensor.matmul(
                        ps[:], xt_sb[:, a, 128 * tb:128 * (tb + 1)],
                        wv_sb[:, a, :], start=(a == 0), stop=(a == KC - 1))
                    if a == 0:
                        pace(m)
                nc.vector.tensor_copy(
                    v_sb[:, tb, :, 0:HD],
                    ps[:].rearrange("p (h d) -> p h d", h=HPC))
            return ("v", tb, emit, 8 * 512 * PE_ROW)

        # scheduler time-fences (ms of sim time): hold each pair's q/k
        # GEMMs in reserve so the out-of-order tile scheduler doesn't burn
        # them early — they are the PE filler for the ACT-paced attention
        # window of the preceding pair
        QK_FENCE = {0: 0.0, 1: 0.0, 2: 0.0, 3: 0.0}

        def qk_unit(j, which, i):
            def emit():
                if which == "q" and j not in q_tiles:
                    q_tiles[j] = qp.tile([128, T], F16, tag="q",
                                         name=f"q{j}")
                wsb = wq_tiles[j] if which == "q" else wk_tiles[j]
                with tc.tile_wait_until(QK_FENCE[j], enable=QK_FENCE[j] > 0):
                    ps = psq.tile([128, 512], F32, tag="acc", name="psqk")
                    for a in range(KC):
                        m = nc.tensor.matmul(
                            ps[:], wsb[:, a, :],
                            xt_sb[:, a, 512 * i:512 * (i + 1)],
                            start=(a == 0), stop=(a == KC - 1))
                        if a == 0:
                            pace(m)
                    if which == "q":
                        nc.vector.tensor_scalar_mul(
                            q_tiles[j][:, 512 * i:512 * (i + 1)], ps[:], SCALE)
                    else:
                        nc.vector.tensor_copy(
                            kT_sb[:, j, 512 * i:512 * (i + 1)], ps[:])
            return ("qk", j, emit, 8 * 512 * PE_ROW)

        y_r = y[:].rearrange("(t p) n -> p t n", p=128)
        part_tiles = {}

        def partial_unit(tb, half):
            # pairs 0-2 of the K accumulation: ready as soon as att(2) has
            # produced these columns, so these are clean att(2)/att(3) fillers
            def emit():
                ps = psq.tile([128, 512], F32, tag="acc", name="psp")
                for j in range(3):
                    m = nc.tensor.matmul(
                        ps[:], ci_tiles[j][:, 128 * tb:128 * (tb + 1)],
                        wp_sb[:, j, 512 * half:512 * (half + 1)],
                        start=(j == 0), stop=(j == 2))
                    if j == 0:
                        pace(m)
                part_tiles[(tb, half)] = partp.tile(
                    [128, 512], F16, tag="part", name=f"part{tb}_{half}")
                nc.vector.tensor_copy(part_tiles[(tb, half)][:], ps[:])
            return ("part", tb, emit, 3 * 512 * PE_ROW)

        def proj_unit(tb, half):
            def emit():
                # late units borrow a (by then idle) score-pool bank so four
                # accumulation groups can be in flight instead of two
                if tb >= 12 and (tb + half) % 2 == 1:
                    ps2t = ps2.tile([128, 2, 512], F32, tag="s2", name="psp2")
                    ps = ps2t[:, 0, :]
                else:
                    ps = psq.tile([128, 512], F32, tag="acc", name="psj")[:]
                for j in range(NPAIR):
                    m = nc.tensor.matmul(
                        ps, ci_tiles[j][:, 128 * tb:128 * (tb + 1)],
                        wp_sb[:, j, 512 * half:512 * (half + 1)],
                        start=(j == 0), stop=(j == NPAIR - 1))
                    if j == 0:
                        pace(m)
                ysb = ysbp.tile([128, 512], F16, tag="ysb", name="ysb")
                nc.vector.tensor_copy(ysb[:], ps)
                yeng = nc.scalar if tb >= 12 else nc.sync
                yeng.dma_start(
                    out=y_r[:, tb, 512 * half:512 * (half + 1)], in_=ysb[:])
            return ("proj", tb, emit, NPAIR * 512 * PE_ROW)

        def finish_unit(tb, half):
            def emit():
                ps = psq.tile([128, 512], F32, tag="acc", name="psf")
                m = nc.tensor.matmul(
                    ps[:], ci_tiles[3][:, 128 * tb:128 * (tb + 1)],
                    wp_sb[:, 3, 512 * half:512 * (half + 1)],
                    start=True, stop=True)
                pace(m)
                ysb = ysbp.tile([128, 512], F16, tag="ysb", name="ysb")
                nc.vector.tensor_add(
                    ysb[:], part_tiles[(tb, half)][:], ps[:])
                nc.sync.dma_start(
                    out=y_r[:, tb, 512 * half:512 * (half + 1)], in_=ysb[:])
            return ("fin", tb, emit, 512 * PE_ROW)

        fillers = deque()
        markers = {}
        for i in range(QBS):
            for tb in range(4 * i, 4 * (i + 1)):
                fillers.append(v_unit(tb))
            fillers.append(qk_unit(0, "q", i))
            fillers.append(qk_unit(0, "k", i))
            markers[(0, i)] = len(fillers)
        for j in range(1, NPAIR):
            for i in range(QBS):
                fillers.append(qk_unit(j, "q", i))
                fillers.append(qk_unit(j, "k", i))
                markers[(j, i)] = len(fillers)
        for tb in range(TB):
            for half in range(2):
                fillers.append(proj_unit(tb, half))
        n_popped = [0]

        def head_legal():
            kind, tb, _, _ = fillers[0]
            if kind == "qk" and tb not in wq_tiles:
                return False  # pair weights not prefetched yet
            if kind == "proj":
                # proj tb needs every pair's attnT columns for its q-block
                return (tb // 4) in clk["att3_done"]
            return True

        def pop_one():
            _, _, emit, pe_ns = fillers.popleft()
            emit()
            n_popped[0] += 1
            clk["pe"] += pe_ns

        def pop_until(target, cap=12):
            n = 0
            while fillers and n < cap and clk["pe"] < target and head_legal():
                pop_one()
                n += 1

        def force_through(marker):
            while n_popped[0] < marker:
                pop_one()

        # ---------------- attention ----------------
        EXP_LAT = 300.0

        def att_qb(j, qb):
            aug = paug.tile([128, 2, 4, 128], F32, tag="aug", name="aug")
            nchunks = 4 * (qb + 1)
            pending = None
            pending_ready = 0.0
            for c in range(nchunks):
                diag = c >= 4 * qb
                o = 128 * (c - 4 * qb) if diag else 0
                s0 = c - 4 * qb
                s2 = ps2.tile([128, 2, 512], F32, tag="s2", name="s2")
                with tc.high_priority():
                    for hh in range(2):
                        m = nc.tensor.matmul(
                            s2[:, hh, o:512],
                            kT_sb[64 * hh:64 * hh + 64, j,
                                  128 * c:128 * (c + 1)],
                            q_tiles[j][64 * hh:64 * hh + 64,
                                       512 * qb + o:512 * (qb + 1)],
                            start=True, stop=True)
                        if hh == 0:
                            anchor[0] = m.ins
                    rows = 2 * (512 - o)
                    clk["pe"] += rows * PE_ROW
                    aT = atp.tile([128, 2, 512], F16, tag="aT", name="aT")
                    nc.scalar.activation(aT[:, :, o:512], s2[:, :, o:512],
                                         AF.Exp)
                    clk["act"] = max(clk["act"], clk["pe"] + EXP_LAT) \
                        + rows * ACT_ROW + ACT_FIX
                    if diag:
                        nc.vector.tensor_mul(
                            aT[:, :, o:o + 128], aT[:, :, o:o + 128],
                            mask_sb[:])
                # flush the previous chunk's a@V now that this chunk's scores
                # are in flight; insert fillers if the PE would beat the exp
                if pending is not None:
                    clk["chunk"] += 1
                    cad = 1 if j == 3 else 5
                    if clk["chunk"] % cad == 0:
                        pop_until(clk["pe"] + 1.0, cap=2)
                    pending()
                pending = _make_av(j, qb, c, diag, s0, aug, aT)
                pending_ready = clk["act"]
            pending()

        def _make_av(j, qb, c, diag, s0, aug, aT):
            def emit_av():
                # masked diagonal subblock s0 goes last so the other a@V
                # matmuls don't sit behind the DVE mask in PE order.
                # start=True zeroes the whole 2KB PSUM bank (zero region), so
                # only the FIRST matmul touching each hh-bank of a fresh aug
                # tile carries it; siblings are ordered after the zeroing and
                # accumulate onto zeros.
                subs = list(range(max(0, s0) + 1, 4)) + [max(0, s0)]
                stk3 = tc.high_priority()
                stk3.__enter__()
                for hh in range(2):
                    clear_mm = None
                    for s in subs:
                        mm = nc.tensor.matmul(
                            aug[:, hh, s, 0:HD + 1],
                            aT[:, hh, 128 * s:128 * (s + 1)],
                            v_sb[:, c, 2 * j + hh, :],
                            start=(c == 0 and clear_mm is None),
                            stop=(c == 4 * qb + s),
                            skip_group_check=True)
                        if c == 0:
                            if clear_mm is None:
                                clear_mm = mm
                            else:
                                add_dep_helper(
                                    mm.ins, clear_mm.ins, sync=True,
                                    reason="PSUM zero-region ordering")
                clk["pe"] += len(subs) * 2 * 65 * PE_ROW
                stk3.__exit__(None, None, None)
                if diag:
                    # stage the finished subblock out of PSUM with one small
                    # DVE copy (shrinks the aug-tile WAR window to ~260ns),
                    # then normalize on the otherwise-idle GpSimd engine
                    with tc.high_priority():
                        stg = rcpp.tile([128, 2, HD + 1], F32, tag="stg",
                                        name="stg")
                        nc.vector.tensor_copy(stg[:], aug[:, :, s0, 0:HD + 1])
                    nb = nbp.tile([128, 2, HD], F16, tag="nb", name="nb")
                    for hh in range(2):
                        nc.gpsimd.normalize_recip(
                            nb[:, hh, :], stg[:, hh, 0:HD],
                            stg[:, hh, HD:HD + 1])
                    # the last pair's late transposes go out on the
                    # Activation engine's DMA queue: its exp stream is done
                    # by then, and this keeps their semaphore waits from
                    # head-of-line blocking the projection drains' y-DMAs
                    # behind them in SP's in-order issue queue
                    eng = nc.scalar if (j == 3 and qb >= 2) else nc.sync
                    eng.dma_start_transpose(
                        ci_sb[:, 512 * qb + 128 * s0:
                              512 * qb + 128 * (s0 + 1)],
                        nb[:])
            return emit_av

        for j in range(NPAIR):
            ci_sb = cisp.tile([128, T], F16, tag="ci", name=f"ci{j}")
            ci_tiles[j] = ci_sb
            fetch_wqk(j + 1)
            fetch_wqk(j + 2)
            fetch_wqk(j + 3)
            clk["pe"] = clk["act"] = 0.0
            for qb in range(QBS):
                force_through(markers[(j, qb)])
                att_qb(j, qb)
                if j == 1:
                    clk["att1_cols"] = 512 * (qb + 1)
                if j == 3:
                    clk["att3_done"].add(qb)

        while fillers:
            pop_one()


_NC_CACHE = None


def _get_nc():
    global _NC_CACHE
    if _NC_CACHE is None:
        _NC_CACHE = build_nc()
    return _NC_CACHE


def _mask_np():
    # mask[kv', hh, q'] = 1 where q' >= kv' (within-chunk causal triangle),
    # duplicated over the two heads packed per score tile
    kv = np.arange(128)[:, None]
    q = np.arange(128)[None, :]
    tri = (q >= kv).astype(np.float16)
    return np.ascontiguousarray(
        np.broadcast_to(tri[:, None, :], (128, 2, 128)))


def shard_inputs(x, w_qkv, w_proj):
    x = np.asarray(x, dtype=np.float16)
    w_qkv = np.asarray(w_qkv, dtype=np.float16)
    w_proj = np.asarray(w_proj, dtype=np.float16)
    mask = _mask_np()
    in_maps = []
    for core in range(N_CORES):
        pair, rank = divmod(core, 2)
        c0 = HD * HPC * rank  # 0 or 512: this core's head-column offset
        in_maps.append({
            "xt": np.ascontiguousarray(x[pair].T),
            "wq": np.ascontiguousarray(w_qkv[:, c0:c0 + 512]),
            "wk": np.ascontiguousarray(w_qkv[:, C + c0:C + c0 + 512]),
            "wv": np.ascontiguousarray(w_qkv[:, 2 * C + c0:2 * C + c0 + 512]),
            "wp": np.ascontiguousarray(w_proj[:, 512 * rank:512 * rank + 512]),
            "mask": mask,
        })
    return in_maps


def assemble_output(results):
    out = np.empty((B, T, C), dtype=np.float32)
    for core in range(N_CORES):
        pair, rank = divmod(core, 2)
        out[pair][:, 512 * rank:512 * rank + 512] = \
            results[core]["y"].astype(np.float32)
    return out


# --- cached PJRT runner (same path run_bass_kernel_spmd takes under axon,
# but keeps the jitted executable so repeat calls skip re-tracing) ---
_RUNNER_CACHE = None


def _make_runner(nc):
    import jax
    import numpy as _np
    from jax.sharding import Mesh, PartitionSpec
    from jax.experimental.shard_map import shard_map
    from concourse import bass2jax
    from concourse.bass2jax import _bass_exec_p, install_neuronx_cc_hook

    install_neuronx_cc_hook()
    part_name = (nc.partition_id_tensor.name
                 if nc.partition_id_tensor else None)
    in_names, out_names, out_avals, zero_shapes = [], [], [], []
    for alloc in nc.m.functions[0].allocations:
        if not isinstance(alloc, mybir.MemoryLocationSet):
            continue
        name = alloc.memorylocations[0].name
        if alloc.kind == "ExternalInput":
            if name != part_name:
                in_names.append(name)
        elif alloc.kind == "ExternalOutput":
            out_names.append(name)
            shape = tuple(alloc.tensor_shape)
            dtype = mybir.dt.np(alloc.dtype)
            out_avals.append(jax.core.ShapedArray(shape, dtype))
            zero_shapes.append((shape, dtype))
    n_params = len(in_names)
    n_outs = len(out_names)
    all_in_names = in_names + out_names
    if part_name is not None:
        all_in_names = all_in_names + [part_name]

    def _body(*args):
        operands = list(args)
        if part_name is not None:
            operands.append(bass2jax.partition_id_tensor())
        outs = _bass_exec_p.bind(
            *operands,
            out_avals=tuple(out_avals),
            in_names=tuple(all_in_names),
            out_names=tuple(out_names),
            lowering_input_output_aliases=(),
            sim_require_finite=True,
            sim_require_nnan=True,
            nc=nc,
        )
        return tuple(outs)

    devices = jax.devices()[:N_CORES]
    mesh = Mesh(_np.asarray(devices), ("core",))
    in_specs = (PartitionSpec("core"),) * (n_params + n_outs)
    out_specs = (PartitionSpec("core"),) * n_outs
    donate = tuple(range(n_params, n_params + n_outs))
    sharded = jax.jit(
        shard_map(_body, mesh=mesh, in_specs=in_specs, out_specs=out_specs,
                  check_rep=False),
        donate_argnums=donate, keep_unused=True,
    )

    def run(in_maps):
        concat_in = [
            _np.concatenate([_np.asarray(in_maps[c][nm]) for c in
                             range(N_CORES)], axis=0)
            for nm in in_names
        ]
        concat_zeros = [
            _np.zeros((N_CORES * s[0], *s[1:]), d) for s, d in zero_shapes
        ]
        out_arrs = sharded(*concat_in, *concat_zeros)
        return [
            {nm: _np.asarray(out_arrs[i]).reshape(
                N_CORES, *out_avals[i].shape)[c]
             for i, nm in enumerate(out_names)}
            for c in range(N_CORES)
        ]

    run.sharded = sharded
    run.in_names = in_names
    run.zero_shapes = zero_shapes
    run.mesh = mesh
    return run


def _get_runner():
    global _RUNNER_CACHE
    if _RUNNER_CACHE is None:
        _RUNNER_CACHE = _make_runner(_get_nc())
    return _RUNNER_CACHE


def kernel(x, w_qkv, w_proj):
    in_maps = shard_inputs(x, w_qkv, w_proj)
    try:
        results = _get_runner()(in_maps)
    except Exception:
        res = run_bass_kernel_spmd(_get_nc(), in_maps, list(range(N_CORES)))
        results = res.results
    return assemble_output(results)
